# revision 21
# baseline (speedup 1.0000x reference)
"""Trainium2 Bass kernel for nn_CTRPredictor (gnn_message_passing).

score[e] = dot(normalize(x[src[e]]), normalize(x[dst[e]]))  for E edges.

Strategy (8 NeuronCores, SPMD):
  - Edges sharded: core i gets edges [i*80000, (i+1)*80000).
  - Each core L2-normalizes its 12500-node slice of x (ACT square, DVE
    reduce, sqrt, reciprocal, scale) to bf16; two half AllGathers replicate
    the normalized table to every core as 4 banks of 25000 rows (gathers on
    the first two banks overlap the second collective).
  - Host groups each core's edges by (src_bank, dst_bank) into 16 groups
    (so bank-local indices fit dma_gather's int16) with a fixed padded
    capacity per group (pad slots gather row 0 and are discarded).
  - Per group: dma_gather x_norm[src] and x_norm[dst] rows (256B bf16)
    across 4 SWDGE queues, DVE bf16 multiply + grouped reduce -> scores.
  - Host un-permutes scores back to edge order.
"""

import numpy as np

N = 100000
D = 128
E = 640000
CORES = 8
EPC = E // CORES          # 80000 edges per core
SLICE = N // CORES        # 12500 nodes normalized per core
QSL = SLICE // 4          # 3125-row quarter slices (AllGather chunks)
QCOL = 25                 # row-columns per quarter in the normalize layout
NBANK = 4
BANK = N // NBANK         # 25000 rows per stripe bank
NGRP = NBANK * NBANK      # 16 (src_bank, dst_bank) groups
GCAP = 5376               # padded edge capacity per group (42*128)
NCALLG = 2                # gather calls per group per endpoint
GCALL = GCAP // NCALLG    # 2688 indices per dma_gather call
CCOL = GCALL // 128       # 21 gathered row-columns per call
ICOL = GCALL // 16        # 168 index columns per call
NCALL = NGRP * NCALLG     # 32 slot-range calls (each does src + dst)
SCOL = NGRP * GCAP // 128  # 672 score columns
SP_NORM = 125             # partitions used in the normalize phase
RN = SLICE // SP_NORM     # 100 rows per partition in normalize phase

_CACHE = {}
LAST_RESULTS = None
RUN_KWARGS = {}  # extra kwargs for run_bass_kernel_spmd (used by test harness)


def _build():
    from concourse import bass, bacc, tile, mybir

    f32 = mybir.dt.float32
    bf16 = mybir.dt.bfloat16
    i16 = mybir.dt.int16
    i32 = mybir.dt.int32

    nc = bacc.Bacc("TRN2", target_bir_lowering=False, debug=False,
                   num_devices=CORES, num_swdge_queues=4,
                   dynamic_dma_scratch_size=49152)

    xsl_d = nc.dram_tensor("xsl", [SP_NORM, RN * D], f32, kind="ExternalInput")
    sidx_d = nc.dram_tensor("src_idx", [128, NCALL * ICOL], i16,
                            kind="ExternalInput")
    didx_d = nc.dram_tensor("dst_idx", [128, NCALL * ICOL], i16,
                            kind="ExternalInput")
    cnt_d = nc.dram_tensor("cnt", [1, NCALL], i32, kind="ExternalInput")
    out_d = nc.dram_tensor("out", [128, SCOL], f32, kind="ExternalOutput")

    with tile.TileContext(nc) as tc:
        with tc.tile_pool(name="dram", bufs=1, space="DRAM") as dp, \
             tc.tile_pool(name="persist", bufs=1) as pp:

            # ---- index tables + score accumulator ----
            sidx = pp.tile([128, NCALL * ICOL], i16)
            didx = pp.tile([128, NCALL * ICOL], i16)
            cnt = pp.tile([1, NCALL], i32)
            nc.sync.dma_start(out=sidx[:, :], in_=sidx_d.ap())
            nc.sync.dma_start(out=didx[:, :], in_=didx_d.ap())
            nc.sync.dma_start(out=cnt[:, :], in_=cnt_d.ap())
            score = pp.tile([128, SCOL], f32)

            # ---- phase 0: normalize this core's slice to bf16 ----
            # fully per-quarter pipeline: each quarter is loaded,
            # normalized, and AllGathered independently so bank q is
            # available without waiting for quarters > q.
            banks = []
            with tc.tile_pool(name="ph0", bufs=1) as p0, \
                 tc.tile_pool(name="sqp", bufs=2) as sqp:
                xsl = p0.tile([SP_NORM, RN * D], f32)
                ns = p0.tile([SP_NORM, RN], f32)
                rns = p0.tile([SP_NORM, RN], f32)
                ntile = p0.tile([SP_NORM, RN * D], bf16)
                for q in range(4):
                    c0 = q * QCOL
                    xseg = xsl[:, c0 * D:(c0 + QCOL) * D]
                    nc.sync.dma_start(
                        out=xseg,
                        in_=xsl_d.ap()[:, c0 * D:(c0 + QCOL) * D])
                    sq = sqp.tile([SP_NORM, QCOL * D], f32, tag="sq")
                    nc.scalar.activation(
                        out=sq[:, :], in_=xseg,
                        func=mybir.ActivationFunctionType.Square)
                    nc.vector.tensor_reduce(
                        out=ns[:, c0:c0 + QCOL],
                        in_=sq[:, :].rearrange("p (r d) -> p r d", d=D),
                        axis=mybir.AxisListType.X,
                        op=mybir.AluOpType.add,
                    )
                    nc.scalar.activation(
                        out=ns[:, c0:c0 + QCOL], in_=ns[:, c0:c0 + QCOL],
                        func=mybir.ActivationFunctionType.Sqrt)
                    nc.vector.reciprocal(out=rns[:, c0:c0 + QCOL],
                                         in_=ns[:, c0:c0 + QCOL])
                    nc.vector.tensor_mul(
                        out=ntile[:, c0 * D:(c0 + QCOL) * D].rearrange(
                            "p (r d) -> p r d", d=D),
                        in0=xseg.rearrange("p (r d) -> p r d", d=D),
                        in1=rns[:, c0:c0 + QCOL].unsqueeze(-1).to_broadcast(
                            [SP_NORM, QCOL, D]),
                    )
                    agin = dp.tile([QSL, D], bf16, name=f"agin{q}")
                    htab = dp.tile([BANK, D], bf16, name=f"htab{q}",
                                   addr_space="Shared")
                    nc.sync.dma_start(
                        out=agin[:, :].rearrange("(p r) d -> p (r d)",
                                                 p=SP_NORM),
                        in_=ntile[:, c0 * D:(c0 + QCOL) * D],
                    )
                    nc.gpsimd.collective_compute(
                        "AllGather",
                        mybir.AluOpType.bypass,
                        replica_groups=[list(range(CORES))],
                        ins=[agin.opt()],
                        outs=[htab.opt()],
                    )
                    banks.append(htab[:, :])

            # ---- main loop: gathers on 4 queues, DVE dot per call ----
            # process groups in bank-availability order: a group needs banks
            # (a, b), and AllGather c completes before c+1 — order by max
            group_order = sorted(range(NGRP),
                                 key=lambda g: (max(g // NBANK, g % NBANK),
                                                g // NBANK, g % NBANK))
            creg = nc.gpsimd.alloc_register("cnt_reg")
            with tc.tile_pool(name="ga", bufs=6) as ga, \
                 tc.tile_pool(name="gb", bufs=6) as gb:
                qn = 0
                for g in group_order:
                    ba, bb = g // NBANK, g % NBANK
                    for c in range(NCALLG):
                        call = g * NCALLG + c
                        col0 = call * ICOL
                        xs_t = ga.tile([128, CCOL * D], bf16, tag="A")
                        xd_t = gb.tile([128, CCOL * D], bf16, tag="B")
                        nc.gpsimd.reg_load(creg, cnt[0:1, call:call + 1])
                        nc.gpsimd.dma_gather(
                            out_ap=xs_t[:, :].rearrange(
                                "p (c d) -> p c d", d=D),
                            in_ap=banks[ba][:, :],
                            idxs_ap=sidx[:, col0:col0 + ICOL],
                            num_idxs=GCALL, num_idxs_reg=creg, elem_size=D,
                            single_packet=False, queue_num=qn % 4,
                        )
                        qn += 1
                        nc.gpsimd.dma_gather(
                            out_ap=xd_t[:, :].rearrange(
                                "p (c d) -> p c d", d=D),
                            in_ap=banks[bb][:, :],
                            idxs_ap=didx[:, col0:col0 + ICOL],
                            num_idxs=GCALL, num_idxs_reg=creg, elem_size=D,
                            single_packet=False, queue_num=qn % 4,
                        )
                        qn += 1
                        nc.vector.tensor_mul(out=xs_t[:, :], in0=xs_t[:, :],
                                             in1=xd_t[:, :])
                        sc0 = call * CCOL
                        nc.vector.tensor_reduce(
                            out=score[:, sc0:sc0 + CCOL],
                            in_=xs_t[:, :].rearrange("p (c d) -> p c d", d=D),
                            axis=mybir.AxisListType.X,
                            op=mybir.AluOpType.add,
                        )

                nc.sync.dma_start(out=out_d.ap(), in_=score[:, :])

    nc.compile()
    return nc


def _node_map(n):
    """node id -> (bank, bank-local index) for the quarter-AllGather layout.

    Slice-local node j sits at ntile[j % 125, (j // 3125)*25 + (j % 3125)//125]
    => agin_q row (p*25 + rr) = node q*3125 + rr*125 + p of the slice, and
    core r's quarter lands at htab_q rows [r*3125, (r+1)*3125).
    """
    r = n // SLICE
    rem = n - r * SLICE
    q = rem // QSL
    w = rem - q * QSL
    rr = w // SP_NORM
    p = w - rr * SP_NORM
    bank = q
    local = r * QSL + p * QCOL + rr
    return bank, local


def _wrap_idx(flat):
    """[GCALL] int16 -> [128, ICOL] in dma_gather's 16-partition wrap."""
    blk = flat.reshape(ICOL, 16).T  # index i at [i%16, i//16]
    return np.tile(blk, (8, 1))


def _prepare_core(src_l, dst_l):
    """Group one core's edges by bank pair; build index tilings + inverse."""
    sb, sl = _node_map(src_l)
    db, dl = _node_map(dst_l)
    key = sb * NBANK + db
    order = np.argsort(key, kind="stable")
    sizes = np.bincount(key, minlength=NGRP)
    if sizes.max() > GCAP:
        raise ValueError(f"group overflow: {sizes.max()} > {GCAP}")
    if sizes.min() <= GCALL + 128:
        raise ValueError(f"group underflow: {sizes.min()} <= {GCALL + 128}")

    sidx = np.zeros((128, NCALL * ICOL), dtype=np.int16)
    didx = np.zeros((128, NCALL * ICOL), dtype=np.int16)
    counts = np.zeros(NCALL, dtype=np.int32)
    # inverse: score of edge order[...] lives at [row, col] of out tile
    rows = np.empty(EPC, dtype=np.int64)
    cols = np.empty(EPC, dtype=np.int64)
    off = 0
    for g in range(NGRP):
        ids = order[off:off + sizes[g]]
        off += sizes[g]
        # ascending src addresses give the src-side gather descriptors
        # HBM locality (the dst side stays random)
        ids = ids[np.argsort(sl[ids], kind="stable")]
        s_pad = np.full(GCAP, -1, dtype=np.int16)
        d_pad = np.full(GCAP, -1, dtype=np.int16)
        s_pad[:ids.size] = sl[ids]
        d_pad[:ids.size] = dl[ids]
        for c in range(NCALLG):
            call = g * NCALLG + c
            col0 = call * ICOL
            seg = slice(c * GCALL, (c + 1) * GCALL)
            sidx[:, col0:col0 + ICOL] = _wrap_idx(s_pad[seg])
            didx[:, col0:col0 + ICOL] = _wrap_idx(d_pad[seg])
            counts[call] = min(max(int(ids.size) - c * GCALL, 0), GCALL)
        j = np.arange(ids.size)
        rows[ids] = j % 128
        cols[ids] = g * (GCAP // 128) + j // 128
    return sidx, didx, counts, rows, cols


def kernel(x, src, dst):
    global LAST_RESULTS
    from concourse.bass_utils import run_bass_kernel_spmd

    if "nc" not in _CACHE:
        _CACHE["nc"] = _build()
    nc = _CACHE["nc"]

    x32 = np.ascontiguousarray(np.asarray(x, dtype=np.float32))
    src_i = np.asarray(src).astype(np.int64)
    dst_i = np.asarray(dst).astype(np.int64)

    in_maps = []
    inv = []
    for i in range(CORES):
        sidx, didx, counts, rows, cols = _prepare_core(
            src_i[i * EPC:(i + 1) * EPC], dst_i[i * EPC:(i + 1) * EPC])
        inv.append((rows, cols))
        in_maps.append({
            "xsl": np.ascontiguousarray(
                x32[i * SLICE:(i + 1) * SLICE]
                .reshape(4, QCOL, SP_NORM, D).transpose(2, 0, 1, 3)
                .reshape(SP_NORM, RN * D)),
            "src_idx": np.ascontiguousarray(sidx),
            "dst_idx": np.ascontiguousarray(didx),
            "cnt": np.ascontiguousarray(counts.reshape(1, NCALL)),
        })

    res = run_bass_kernel_spmd(nc, in_maps, core_ids=list(range(CORES)),
                               **RUN_KWARGS)
    LAST_RESULTS = res

    out = np.empty(E, dtype=np.float32)
    for i in range(CORES):
        tilev = np.asarray(res.results[i]["out"])
        rows, cols = inv[i]
        out[i * EPC:(i + 1) * EPC] = tilev[rows, cols]
    return out.reshape(E, 1)



# revision 27
# speedup vs baseline: 1.0057x; 1.0057x over previous
"""Trainium2 Bass kernel for nn_CTRPredictor (gnn_message_passing).

score[e] = dot(normalize(x[src[e]]), normalize(x[dst[e]]))  for E edges.

Strategy (8 NeuronCores, SPMD):
  - Edges sharded: core i gets edges [i*80000, (i+1)*80000).
  - Each core L2-normalizes its 12500-node slice of x (ACT square, DVE
    reduce, sqrt, reciprocal, scale) to bf16; two half AllGathers replicate
    the normalized table to every core as 4 banks of 25000 rows (gathers on
    the first two banks overlap the second collective).
  - Host groups each core's edges by (src_bank, dst_bank) into 16 groups
    (so bank-local indices fit dma_gather's int16) with a fixed padded
    capacity per group (pad slots gather row 0 and are discarded).
  - Per group: dma_gather x_norm[src] and x_norm[dst] rows (256B bf16)
    across 4 SWDGE queues, DVE bf16 multiply + grouped reduce -> scores.
  - Host un-permutes scores back to edge order.
"""

import numpy as np

N = 100000
D = 128
E = 640000
CORES = 8
EPC = E // CORES          # 80000 edges per core
SLICE = N // CORES        # 12500 nodes normalized per core
QSL = SLICE // 4          # 3125-row quarter slices (AllGather chunks)
QCOL = 25                 # row-columns per quarter in the normalize layout
NBANK = 4
BANK = N // NBANK         # 25000 rows per stripe bank
NGRP = NBANK * NBANK      # 16 (src_bank, dst_bank) groups
GCAP = 5376               # padded edge capacity per group (42*128)
NCALLG = 2                # gather calls per group per endpoint
GCALL = GCAP // NCALLG    # 2688 indices per dma_gather call
CCOL = GCALL // 128       # 21 gathered row-columns per call
ICOL = GCALL // 16        # 168 index columns per call
NCALL = NGRP * NCALLG     # 32 slot-range calls (each does src + dst)
SCOL = NGRP * GCAP // 128  # 672 score columns
SP_NORM = 125             # partitions used in the normalize phase
RN = SLICE // SP_NORM     # 100 rows per partition in normalize phase

_CACHE = {}
LAST_RESULTS = None
RUN_KWARGS = {}  # extra kwargs for run_bass_kernel_spmd (used by test harness)


def _build():
    from concourse import bass, bacc, tile, mybir

    f32 = mybir.dt.float32
    bf16 = mybir.dt.bfloat16
    i16 = mybir.dt.int16
    i32 = mybir.dt.int32

    nc = bacc.Bacc("TRN2", target_bir_lowering=False, debug=False,
                   num_devices=CORES, num_swdge_queues=4,
                   dynamic_dma_scratch_size=40960)

    xsl_d = nc.dram_tensor("xsl", [SP_NORM, RN * D], f32, kind="ExternalInput")
    sidx_d = nc.dram_tensor("src_idx", [128, NCALL * ICOL], i16,
                            kind="ExternalInput")
    didx_d = nc.dram_tensor("dst_idx", [128, NCALL * ICOL], i16,
                            kind="ExternalInput")
    cnt_d = nc.dram_tensor("cnt", [1, NCALL], i32, kind="ExternalInput")
    out_d = nc.dram_tensor("out", [128, SCOL], f32, kind="ExternalOutput")

    with tile.TileContext(nc) as tc:
        with tc.tile_pool(name="dram", bufs=1, space="DRAM") as dp, \
             tc.tile_pool(name="persist", bufs=1) as pp:

            # ---- index tables + score accumulator ----
            sidx = pp.tile([128, NCALL * ICOL], i16)
            didx = pp.tile([128, NCALL * ICOL], i16)
            cnt = pp.tile([1, NCALL], i32)
            nc.sync.dma_start(out=sidx[:, :], in_=sidx_d.ap())
            nc.sync.dma_start(out=didx[:, :], in_=didx_d.ap())
            nc.sync.dma_start(out=cnt[:, :], in_=cnt_d.ap())
            score = pp.tile([128, SCOL], f32)

            # ---- phase 0: normalize this core's slice to bf16 ----
            # fully per-quarter pipeline: each quarter is loaded,
            # normalized, and AllGathered independently so bank q is
            # available without waiting for quarters > q.
            banks = []
            with tc.tile_pool(name="ph0", bufs=1) as p0, \
                 tc.tile_pool(name="sqp", bufs=2) as sqp:
                xsl = p0.tile([SP_NORM, RN * D], f32)
                ns = p0.tile([SP_NORM, RN], f32)
                rns = p0.tile([SP_NORM, RN], f32)
                ntile = p0.tile([SP_NORM, RN * D], bf16)
                for q in range(4):
                    c0 = q * QCOL
                    xseg = xsl[:, c0 * D:(c0 + QCOL) * D]
                    nc.sync.dma_start(
                        out=xseg,
                        in_=xsl_d.ap()[:, c0 * D:(c0 + QCOL) * D])
                    sq = sqp.tile([SP_NORM, QCOL * D], f32, tag="sq")
                    nc.scalar.activation(
                        out=sq[:, :], in_=xseg,
                        func=mybir.ActivationFunctionType.Square)
                    nc.vector.tensor_reduce(
                        out=ns[:, c0:c0 + QCOL],
                        in_=sq[:, :].rearrange("p (r d) -> p r d", d=D),
                        axis=mybir.AxisListType.X,
                        op=mybir.AluOpType.add,
                    )
                    nc.scalar.activation(
                        out=ns[:, c0:c0 + QCOL], in_=ns[:, c0:c0 + QCOL],
                        func=mybir.ActivationFunctionType.Sqrt)
                    nc.vector.reciprocal(out=rns[:, c0:c0 + QCOL],
                                         in_=ns[:, c0:c0 + QCOL])
                    nc.vector.tensor_mul(
                        out=ntile[:, c0 * D:(c0 + QCOL) * D].rearrange(
                            "p (r d) -> p r d", d=D),
                        in0=xseg.rearrange("p (r d) -> p r d", d=D),
                        in1=rns[:, c0:c0 + QCOL].unsqueeze(-1).to_broadcast(
                            [SP_NORM, QCOL, D]),
                    )
                    agin = dp.tile([QSL, D], bf16, name=f"agin{q}")
                    htab = dp.tile([BANK, D], bf16, name=f"htab{q}",
                                   addr_space="Shared")
                    nc.sync.dma_start(
                        out=agin[:, :].rearrange("(p r) d -> p (r d)",
                                                 p=SP_NORM),
                        in_=ntile[:, c0 * D:(c0 + QCOL) * D],
                    )
                    nc.gpsimd.collective_compute(
                        "AllGather",
                        mybir.AluOpType.bypass,
                        replica_groups=[list(range(CORES))],
                        ins=[agin.opt()],
                        outs=[htab.opt()],
                    )
                    banks.append(htab[:, :])

            # ---- main loop: gathers on 4 queues, DVE dot per call ----
            # process groups in bank-availability order: a group needs banks
            # (a, b), and AllGather c completes before c+1 — order by max
            group_order = sorted(range(NGRP),
                                 key=lambda g: (max(g // NBANK, g % NBANK),
                                                g // NBANK, g % NBANK))
            creg = nc.gpsimd.alloc_register("cnt_reg")
            with tc.tile_pool(name="ga", bufs=7) as ga, \
                 tc.tile_pool(name="gb", bufs=7) as gb:
                qn = 0
                for g in group_order:
                    ba, bb = g // NBANK, g % NBANK
                    for c in range(NCALLG):
                        call = g * NCALLG + c
                        col0 = call * ICOL
                        xs_t = ga.tile([128, CCOL * D], bf16, tag="A")
                        xd_t = gb.tile([128, CCOL * D], bf16, tag="B")
                        nc.gpsimd.reg_load(creg, cnt[0:1, call:call + 1])
                        nc.gpsimd.dma_gather(
                            out_ap=xs_t[:, :].rearrange(
                                "p (c d) -> p c d", d=D),
                            in_ap=banks[ba][:, :],
                            idxs_ap=sidx[:, col0:col0 + ICOL],
                            num_idxs=GCALL, num_idxs_reg=creg, elem_size=D,
                            single_packet=False, queue_num=qn % 4,
                        )
                        qn += 1
                        nc.gpsimd.dma_gather(
                            out_ap=xd_t[:, :].rearrange(
                                "p (c d) -> p c d", d=D),
                            in_ap=banks[bb][:, :],
                            idxs_ap=didx[:, col0:col0 + ICOL],
                            num_idxs=GCALL, num_idxs_reg=creg, elem_size=D,
                            single_packet=False, queue_num=qn % 4,
                        )
                        qn += 1
                        nc.vector.tensor_mul(out=xs_t[:, :], in0=xs_t[:, :],
                                             in1=xd_t[:, :])
                        sc0 = call * CCOL
                        nc.vector.tensor_reduce(
                            out=score[:, sc0:sc0 + CCOL],
                            in_=xs_t[:, :].rearrange("p (c d) -> p c d", d=D),
                            axis=mybir.AxisListType.X,
                            op=mybir.AluOpType.add,
                        )

                nc.sync.dma_start(out=out_d.ap(), in_=score[:, :])

    nc.compile()
    return nc


def _node_map(n):
    """node id -> (bank, bank-local index) for the quarter-AllGather layout.

    Slice-local node j sits at ntile[j % 125, (j // 3125)*25 + (j % 3125)//125]
    => agin_q row (p*25 + rr) = node q*3125 + rr*125 + p of the slice, and
    core r's quarter lands at htab_q rows [r*3125, (r+1)*3125).
    """
    r = n // SLICE
    rem = n - r * SLICE
    q = rem // QSL
    w = rem - q * QSL
    rr = w // SP_NORM
    p = w - rr * SP_NORM
    bank = q
    local = r * QSL + p * QCOL + rr
    return bank, local


def _wrap_idx(flat):
    """[GCALL] int16 -> [128, ICOL] in dma_gather's 16-partition wrap."""
    blk = flat.reshape(ICOL, 16).T  # index i at [i%16, i//16]
    return np.tile(blk, (8, 1))


def _prepare_core(src_l, dst_l):
    """Group one core's edges by bank pair; build index tilings + inverse."""
    sb, sl = _node_map(src_l)
    db, dl = _node_map(dst_l)
    key = sb * NBANK + db
    order = np.argsort(key, kind="stable")
    sizes = np.bincount(key, minlength=NGRP)
    if sizes.max() > GCAP:
        raise ValueError(f"group overflow: {sizes.max()} > {GCAP}")
    if sizes.min() <= GCALL + 128:
        raise ValueError(f"group underflow: {sizes.min()} <= {GCALL + 128}")

    sidx = np.zeros((128, NCALL * ICOL), dtype=np.int16)
    didx = np.zeros((128, NCALL * ICOL), dtype=np.int16)
    counts = np.zeros(NCALL, dtype=np.int32)
    # inverse: score of edge order[...] lives at [row, col] of out tile
    rows = np.empty(EPC, dtype=np.int64)
    cols = np.empty(EPC, dtype=np.int64)
    off = 0
    for g in range(NGRP):
        ids = order[off:off + sizes[g]]
        off += sizes[g]
        # ascending src addresses give the src-side gather descriptors
        # HBM locality (the dst side stays random)
        ids = ids[np.argsort(sl[ids], kind="stable")]
        s_pad = np.full(GCAP, -1, dtype=np.int16)
        d_pad = np.full(GCAP, -1, dtype=np.int16)
        s_pad[:ids.size] = sl[ids]
        d_pad[:ids.size] = dl[ids]
        for c in range(NCALLG):
            call = g * NCALLG + c
            col0 = call * ICOL
            seg = slice(c * GCALL, (c + 1) * GCALL)
            sidx[:, col0:col0 + ICOL] = _wrap_idx(s_pad[seg])
            didx[:, col0:col0 + ICOL] = _wrap_idx(d_pad[seg])
            counts[call] = min(max(int(ids.size) - c * GCALL, 0), GCALL)
        j = np.arange(ids.size)
        rows[ids] = j % 128
        cols[ids] = g * (GCAP // 128) + j // 128
    return sidx, didx, counts, rows, cols


def kernel(x, src, dst):
    global LAST_RESULTS
    from concourse.bass_utils import run_bass_kernel_spmd

    if "nc" not in _CACHE:
        _CACHE["nc"] = _build()
    nc = _CACHE["nc"]

    x32 = np.ascontiguousarray(np.asarray(x, dtype=np.float32))
    src_i = np.asarray(src).astype(np.int64)
    dst_i = np.asarray(dst).astype(np.int64)

    in_maps = []
    inv = []
    for i in range(CORES):
        sidx, didx, counts, rows, cols = _prepare_core(
            src_i[i * EPC:(i + 1) * EPC], dst_i[i * EPC:(i + 1) * EPC])
        inv.append((rows, cols))
        in_maps.append({
            "xsl": np.ascontiguousarray(
                x32[i * SLICE:(i + 1) * SLICE]
                .reshape(4, QCOL, SP_NORM, D).transpose(2, 0, 1, 3)
                .reshape(SP_NORM, RN * D)),
            "src_idx": np.ascontiguousarray(sidx),
            "dst_idx": np.ascontiguousarray(didx),
            "cnt": np.ascontiguousarray(counts.reshape(1, NCALL)),
        })

    res = run_bass_kernel_spmd(nc, in_maps, core_ids=list(range(CORES)),
                               **RUN_KWARGS)
    LAST_RESULTS = res

    out = np.empty(E, dtype=np.float32)
    for i in range(CORES):
        tilev = np.asarray(res.results[i]["out"])
        rows, cols = inv[i]
        out[i * EPC:(i + 1) * EPC] = tilev[rows, cols]
    return out.reshape(E, 1)

